# revision 41
# baseline (speedup 1.0000x reference)
"""Trainium2 Bass kernel for MockHilbertRingAttention.

Math (from the reference):
  mapping = snake-scan permutation of [0, 8192)
  idx     = mapping[0:2048]                (RANK=0, chunk=2048)
  xg[b]   = x[b, idx, :]                      [2, 2048, 2048]
  P[r,b]  = softmax(scores[r,b] / sqrt(128), axis=-1)
  acc[b]  = sum_r P[r,b] @ xg[b]              [2, 2048, 2048]
  out     = zeros(2, 8192, 2048); out[:, idx, :] = acc

Distribution: 8 cores = 4 rings x 2 batches; core c handles (r=c//2, b=c%2).
Each core computes  out_c = softmax(scores[r,b]/T) @ xg[b]  (normalized), the
host sums the 4 ring partials per batch and scatters rows back.

Device kernel per core (2048x2048x2048 fp32 matmul + softmax):
  - scores are fed pre-transposed (j-major) so the exp'd tile can be used
    directly as the matmul stationary operand [K=j, M=i]
  - row sums of exp come from an extra N=1 matmul against a ones vector,
    accumulated in PSUM alongside the main output
  - normalization (1/rowsum) is applied per-partition while draining PSUM
  - matmuls run in float32r mode (full fp32 data, 1 cycle/row at N=512)
"""

import numpy as np

P = 128
CHUNK = 2048
SEQ = 8192
HIDDEN = 2048
RING = 4
BATCH = 2
NCORES = 8
IT = CHUNK // P  # 16 output row blocks
JT = CHUNK // P  # 16 contraction blocks
ND = 4  # 4 moving blocks of 512 over hidden dim
NDW = HIDDEN // ND  # 512
SCALE = float(1.0 / np.sqrt(128.0))

_CACHE = {}
_TRACE = False  # set by test.py to capture HW exec time via NTFF profiling
_LAST_EXEC_NS = None
# build-time knobs for A/B benchmarking (bench.py overrides).
# fp16 matmul operands: ~10% faster than f32r (fast-weight-load on the PE,
# half the DMA bytes) at rel err 2.4e-4 vs 1.2e-4.
_OPTS = {
    "st_queue": "act", "psum_o": 7, "psum_s": 1, "dtype": "fp16",
    "st16": True, "pool_bufs": 2, "no_ones": False, "algo": "merged",
}


def _hilbert_idx() -> np.ndarray:
    """First CHUNK entries of the snake-scan mapping (mapping[pos] = scan idx)."""
    size = SEQ
    grid = int(np.ceil(np.sqrt(size)))
    order = []
    for row in range(grid):
        cols = range(grid) if row % 2 == 0 else range(grid - 1, -1, -1)
        for col in cols:
            pos = row * grid + col
            if pos < size:
                order.append(pos)
    mapping = np.zeros(size, dtype=np.int64)
    mapping[np.asarray(order)] = np.arange(size, dtype=np.int64)
    return mapping[:CHUNK]


def _build_nc(reps: int = 1, reload_xg: bool = False):
    # reps>1 repeats the whole compute body (bench-only: isolates steady-state
    # device throughput from per-dispatch overhead). reload_xg puts the xg
    # resident-load inside the rep loop so each rep pays the pipe-fill the
    # real single-shot invocation pays.
    import concourse.mybir as mybir
    import concourse.tile as tile
    from concourse import bacc

    f32 = mybir.dt.float32
    f32r = mybir.dt.float32r
    f16 = mybir.dt.float16
    mm_dt = f16 if _OPTS["dtype"] == "fp16" else f32r

    nc = bacc.Bacc(
        "TRN2", target_bir_lowering=False, debug=False, num_devices=NCORES
    )
    # xg is declared in the matmul dtype: f32r (same bits as fp32, PE rounds
    # on ingest; the BIR verifier requires fp32r-typed producers) or fp16
    # (host-cast, gets fast-weight-load on the PE).
    # st layout [it, j%128, jt, i]: per-partition-contiguous for the slab DMA.
    st_dt = f16 if _OPTS["st16"] else f32
    st = nc.dram_tensor("st", [IT, P, JT * P], st_dt, kind="ExternalInput").ap()
    xg = nc.dram_tensor("xg", [CHUNK, HIDDEN], mm_dt, kind="ExternalInput").ap()
    out = nc.dram_tensor("out", [CHUNK, HIDDEN], f32, kind="ExternalOutput").ap()

    with tile.TileContext(nc) as tc:
        with (
            tc.tile_pool(name="xpool", bufs=1) as xpool,
            tc.tile_pool(name="spool", bufs=_OPTS["pool_bufs"]) as spool,
            tc.tile_pool(name="ppool", bufs=_OPTS["pool_bufs"]) as ppool,
            tc.tile_pool(name="opool", bufs=2) as opool,
            tc.tile_pool(name="cpool", bufs=1) as cpool,
            tc.tile_pool(name="psum_o", bufs=_OPTS["psum_o"], space="PSUM") as psum_o,
            tc.tile_pool(name="psum_s", bufs=_OPTS["psum_s"], space="PSUM") as psum_s,
        ):
            # fp32r matmuls need a moving free dim >= 2, so the rowsum "ones"
            # vector is two columns wide; for f32r it is built via exp(0*x)
            # since neither memset nor DMA can emit the f32r dtype the
            # verifier wants.
            if mm_dt == f32r:
                zeros_t = cpool.tile([P, 2], f32, name="zeros_t", tag="zeros")
                nc.vector.memset(zeros_t[:], 0.0)
                ones_t = cpool.tile([P, 2], f32r, name="ones_t", tag="ones")
                nc.scalar.activation(
                    ones_t[:], zeros_t[:], mybir.ActivationFunctionType.Exp,
                    scale=0.0,
                )
            else:
                ones_t = cpool.tile([P, 2], mm_dt, name="ones_t", tag="ones")
                nc.vector.memset(ones_t[:], 1.0)

            # xg stays resident in SBUF: 16 tiles of [128, 2048] (128 KiB/partition)
            xg_tiles = []

            def load_xg():
                xg_tiles.clear()
                for jt in range(JT):
                    xt = xpool.tile(
                        [P, HIDDEN], mm_dt, name=f"xg_{jt}", tag=f"xg_{jt}"
                    )
                    nc.sync.dma_start(xt[:], xg[jt * P : (jt + 1) * P, :])
                    xg_tiles.append(xt)

            load_xg()

            schedule = []
            for rep in range(reps):
                for it in range(IT):
                    schedule.append((rep, it))
            for rep, it in schedule:
                if reload_xg and it == 0 and rep > 0:
                    load_xg()
                # load scores^T slab for this row block: [j-part, (jt, i)].
                # ACT's HWDGE queue, so it isn't serialized behind the 16.8MB
                # xg load on SP's queue at kernel start.
                st_t = spool.tile([P, JT * P], st_dt, name="st_t", tag="st")
                dma_eng = nc.scalar if _OPTS["st_queue"] == "act" else nc.sync
                dma_eng.dma_start(st_t[:], st[it])
                # P^T = exp(scores^T / sqrt(head_dim)), rounded to mm_dt by ACT
                pt_t = ppool.tile([P, JT * P], mm_dt, name="pt_t", tag="pt")
                nc.scalar.activation(
                    pt_t[:], st_t[:], mybir.ActivationFunctionType.Exp, scale=SCALE
                )

                ps_s = psum_s.tile([P, 2], f32, name="ps_s", tag="ps_s")
                ps_tiles = [
                    psum_o.tile([P, NDW], f32, name=f"ps_o{db}", tag="ps_o")
                    for db in range(ND)
                ]
                for jt in range(JT):
                    lhsT = pt_t[:, jt * P : (jt + 1) * P]
                    start = jt == 0
                    stop = jt == JT - 1
                    for db in range(ND):
                        nc.tensor.matmul(
                            ps_tiles[db][:],
                            lhsT,
                            xg_tiles[jt][:, db * NDW : (db + 1) * NDW],
                            start=start,
                            stop=stop,
                        )
                    if not _OPTS["no_ones"]:
                        nc.tensor.matmul(
                            ps_s[:],
                            lhsT,
                            ones_t[:],
                            start=start,
                            stop=stop,
                        )

                rec = cpool.tile([P, 1], f32, name="rec", tag="rec", bufs=2)
                nc.vector.reciprocal(rec[:], ps_s[:, 0:1])
                ot = opool.tile([P, HIDDEN], f32, name="ot", tag="ot")
                for db in range(ND):
                    nc.vector.tensor_scalar_mul(
                        ot[:, db * NDW : (db + 1) * NDW], ps_tiles[db][:], rec[:]
                    )
                nc.sync.dma_start(out[it * P : (it + 1) * P, :], ot[:])

    nc.compile()
    return nc


def _build_nc_merged(reps: int = 1):
    """Merged-rings algorithm: sum_r softmax_r @ X == (sum_r softmax_r) @ X.

    Cores = 2 batches x 4 j-quarters. Each core exps all 4 rings in natural
    layout (accum_out gives the softmax row-sums for free), builds the merged
    normalized W for its j-quarter (columns 0-511 after a host-side column
    rotation), PE-transposes W, and matmuls against its 512-row slice of xg.
    Host sums the 4 quarter-partials per batch. 4x fewer matmul FLOPs than
    the per-ring scheme.
    """
    import concourse.mybir as mybir
    import concourse.tile as tile
    from concourse import bacc
    from concourse.masks import make_identity

    f32 = mybir.dt.float32
    f16 = mybir.dt.float16
    QW = 512  # j-quarter width
    QT = QW // P  # 4 contraction tiles

    nc = bacc.Bacc(
        "TRN2", target_bir_lowering=False, debug=False, num_devices=NCORES
    )
    stn = nc.dram_tensor(
        "stn", [RING, IT, P, CHUNK], f16, kind="ExternalInput"
    ).ap()
    xgq = nc.dram_tensor("xgq", [QW, HIDDEN], f16, kind="ExternalInput").ap()
    out = nc.dram_tensor("out", [CHUNK, HIDDEN], f32, kind="ExternalOutput").ap()

    with tile.TileContext(nc) as tc:
        with (
            tc.tile_pool(name="xpool", bufs=1) as xpool,
            tc.tile_pool(name="spool", bufs=3) as spool,
            tc.tile_pool(name="epool", bufs=2) as epool,
            tc.tile_pool(name="wpool", bufs=2) as wpool,
            tc.tile_pool(name="wtpool", bufs=2) as wtpool,
            tc.tile_pool(name="opool", bufs=2) as opool,
            tc.tile_pool(name="cpool", bufs=1) as cpool,
            tc.tile_pool(name="sapool", bufs=8) as sapool,
            tc.tile_pool(name="psum_o", bufs=6, space="PSUM") as psum_o,
            tc.tile_pool(name="psum_t", bufs=2, space="PSUM") as psum_t,
        ):
            ident = cpool.tile([P, P], f16, name="ident", tag="ident")
            make_identity(nc, ident[:])

            xg_tiles = []
            for jt in range(QT):
                xt = xpool.tile([P, HIDDEN], f16, name=f"xgq_{jt}", tag=f"xgq_{jt}")
                nc.sync.dma_start(xt[:], xgq[jt * P : (jt + 1) * P, :])
                xg_tiles.append(xt)

            for it in [i for _ in range(reps) for i in range(IT)]:
                # all 4 ring slabs DMA'd from the SP queue up front so the ACT
                # stream is pure back-to-back exps (ACT is the bottleneck)
                st_ts = []
                for r in range(RING):
                    st_t = spool.tile(
                        [P, CHUNK], f16, name=f"st_t{r}", tag=f"st{r}"
                    )
                    nc.sync.dma_start(st_t[:], stn[r, it])
                    st_ts.append(st_t)
                sa4 = sapool.tile([P, RING], f32, name="sa4", tag="sa4")
                e_ts = []
                for r in range(RING):
                    e_t = epool.tile([P, CHUNK], f16, name=f"e_t{r}", tag=f"e{r}")
                    nc.scalar.activation(
                        e_t[:], st_ts[r][:], mybir.ActivationFunctionType.Exp,
                        scale=SCALE, accum_out=sa4[:, r : r + 1],
                    )
                    e_ts.append(e_t)
                rec4 = sapool.tile([P, RING], f32, name="rec4", tag="rec4")
                nc.vector.reciprocal(rec4[:], sa4[:])
                w_t = wpool.tile([P, QW], f16, name="w_t", tag="w")
                for r in range(RING):
                    rec_r = rec4[:, r : r + 1]
                    if r == 0:
                        nc.vector.tensor_scalar_mul(
                            w_t[:], e_ts[r][:, 0:QW], rec_r
                        )
                    else:
                        tmp = wpool.tile([P, QW], f16, name="tmp", tag="tmp")
                        nc.vector.tensor_scalar_mul(
                            tmp[:], e_ts[r][:, 0:QW], rec_r
                        )
                        nc.vector.tensor_tensor(
                            w_t[:], w_t[:], tmp[:], op=mybir.AluOpType.add
                        )

                # W^T tiles for the matmul stationary side
                wt_tiles = []
                for jt in range(QT):
                    pst = psum_t.tile([P, P], f16, name="pst", tag="pst")
                    nc.tensor.transpose(
                        pst[:], w_t[:, jt * P : (jt + 1) * P], ident[:]
                    )
                    wt = wtpool.tile([P, P], f16, name=f"wt{jt}", tag=f"wt{jt}")
                    nc.vector.tensor_copy(wt[:], pst[:])
                    wt_tiles.append(wt)

                ps_tiles = [
                    psum_o.tile([P, NDW], f32, name=f"ps_o{db}", tag="ps_o")
                    for db in range(ND)
                ]
                for jt in range(QT):
                    for db in range(ND):
                        nc.tensor.matmul(
                            ps_tiles[db][:],
                            wt_tiles[jt][:],
                            xg_tiles[jt][:, db * NDW : (db + 1) * NDW],
                            start=(jt == 0),
                            stop=(jt == QT - 1),
                        )
                ot = opool.tile([P, HIDDEN], f32, name="ot", tag="ot")
                for db in range(ND):
                    # DVE, not ACT: the four exp passes saturate ACT
                    nc.vector.tensor_copy(
                        ot[:, db * NDW : (db + 1) * NDW], ps_tiles[db][:]
                    )
                nc.sync.dma_start(out[it * P : (it + 1) * P, :], ot[:])

    nc.compile()
    return nc


def _prep_inputs_ring(x, scores, idx):
    xg = x[:, idx, :]
    if _OPTS["dtype"] == "fp16":
        xg = xg.astype(np.float16)
    else:
        xg = np.ascontiguousarray(xg)
    # blocked transpose of scores for j-major device layout:
    # stb[r, b, it, p, jt*128+i] = scores[r, b, it*128+i, jt*128+p]
    stb_t = scores.reshape(RING, BATCH, IT, P, JT, P).transpose(0, 1, 2, 5, 4, 3)
    stb = stb_t.astype(np.float16 if _OPTS["st16"] else np.float32).reshape(
        RING, BATCH, IT, P, JT * P
    )
    return [{"st": stb[c // 2, c % 2], "xg": xg[c % 2]} for c in range(NCORES)]


def _prep_inputs_merged(x, scores, idx):
    QW = 512
    xg = x[:, idx, :]
    in_maps = []
    for c in range(NCORES):
        b, q = c // 4, c % 4
        s_b = scores[:, b]  # [4, 2048, 2048]
        # rotate j so this core's quarter occupies columns 0..511 (softmax is
        # column-order invariant; the matmul only touches the first quarter)
        stn = np.empty((RING, CHUNK, CHUNK), np.float16)
        k = q * QW
        stn[..., : CHUNK - k] = s_b[..., k:]
        if k:
            stn[..., CHUNK - k :] = s_b[..., :k]
        in_maps.append({
            "stn": stn.reshape(RING, IT, P, CHUNK),
            "xgq": xg[b][k : k + QW].astype(np.float16),
        })
    return in_maps


def kernel(**inputs) -> np.ndarray:
    from concourse import bass_utils

    x = np.asarray(inputs["x"], dtype=np.float32)
    scores = np.asarray(inputs["scores"], dtype=np.float32)

    idx = _hilbert_idx()
    merged = _OPTS["algo"] == "merged"
    if "nc" not in _CACHE:
        _CACHE["nc"] = _build_nc_merged() if merged else _build_nc()
    nc = _CACHE["nc"]

    in_maps = (
        _prep_inputs_merged(x, scores, idx)
        if merged
        else _prep_inputs_ring(x, scores, idx)
    )
    res = bass_utils.run_bass_kernel_spmd(
        nc, in_maps, core_ids=list(range(NCORES)), trace=_TRACE
    )
    if _TRACE:
        global _LAST_EXEC_NS
        _LAST_EXEC_NS = res.exec_time_ns

    acc = np.zeros((BATCH, CHUNK, HIDDEN), dtype=np.float32)
    for c in range(NCORES):
        acc[c // 4 if merged else c % 2] += res.results[c]["out"]

    out_full = np.zeros((BATCH, SEQ, HIDDEN), dtype=np.float32)
    out_full[:, idx, :] = acc
    return out_full


# revision 45
# speedup vs baseline: 1.5786x; 1.5786x over previous
"""Trainium2 Bass kernel for MockHilbertRingAttention.

Math (from the reference):
  mapping = snake-scan permutation of [0, 8192)
  idx     = mapping[0:2048]                (RANK=0, chunk=2048)
  xg[b]   = x[b, idx, :]                      [2, 2048, 2048]
  P[r,b]  = softmax(scores[r,b] / sqrt(128), axis=-1)
  acc[b]  = sum_r P[r,b] @ xg[b]              [2, 2048, 2048]
  out     = zeros(2, 8192, 2048); out[:, idx, :] = acc

Distribution: 8 cores = 4 rings x 2 batches; core c handles (r=c//2, b=c%2).
Each core computes  out_c = softmax(scores[r,b]/T) @ xg[b]  (normalized), the
host sums the 4 ring partials per batch and scatters rows back.

Device kernel per core (2048x2048x2048 fp32 matmul + softmax):
  - scores are fed pre-transposed (j-major) so the exp'd tile can be used
    directly as the matmul stationary operand [K=j, M=i]
  - row sums of exp come from an extra N=1 matmul against a ones vector,
    accumulated in PSUM alongside the main output
  - normalization (1/rowsum) is applied per-partition while draining PSUM
  - matmuls run in float32r mode (full fp32 data, 1 cycle/row at N=512)
"""

import numpy as np

P = 128
CHUNK = 2048
SEQ = 8192
HIDDEN = 2048
RING = 4
BATCH = 2
NCORES = 8
IT = CHUNK // P  # 16 output row blocks
JT = CHUNK // P  # 16 contraction blocks
ND = 4  # 4 moving blocks of 512 over hidden dim
NDW = HIDDEN // ND  # 512
SCALE = float(1.0 / np.sqrt(128.0))

_CACHE = {}
_TRACE = False  # set by test.py to capture HW exec time via NTFF profiling
_LAST_EXEC_NS = None
# build-time knobs for A/B benchmarking (bench.py overrides).
# fp16 matmul operands: ~10% faster than f32r (fast-weight-load on the PE,
# half the DMA bytes) at rel err 2.4e-4 vs 1.2e-4.
_OPTS = {
    "st_queue": "act", "psum_o": 7, "psum_s": 1, "dtype": "fp16",
    "st16": True, "pool_bufs": 2, "no_ones": False, "algo": "merged",
    "out16": True,
}


def _hilbert_idx() -> np.ndarray:
    """First CHUNK entries of the snake-scan mapping (mapping[pos] = scan idx)."""
    size = SEQ
    grid = int(np.ceil(np.sqrt(size)))
    order = []
    for row in range(grid):
        cols = range(grid) if row % 2 == 0 else range(grid - 1, -1, -1)
        for col in cols:
            pos = row * grid + col
            if pos < size:
                order.append(pos)
    mapping = np.zeros(size, dtype=np.int64)
    mapping[np.asarray(order)] = np.arange(size, dtype=np.int64)
    return mapping[:CHUNK]


def _build_nc(reps: int = 1, reload_xg: bool = False):
    # reps>1 repeats the whole compute body (bench-only: isolates steady-state
    # device throughput from per-dispatch overhead). reload_xg puts the xg
    # resident-load inside the rep loop so each rep pays the pipe-fill the
    # real single-shot invocation pays.
    import concourse.mybir as mybir
    import concourse.tile as tile
    from concourse import bacc

    f32 = mybir.dt.float32
    f32r = mybir.dt.float32r
    f16 = mybir.dt.float16
    mm_dt = f16 if _OPTS["dtype"] == "fp16" else f32r

    nc = bacc.Bacc(
        "TRN2", target_bir_lowering=False, debug=False, num_devices=NCORES
    )
    # xg is declared in the matmul dtype: f32r (same bits as fp32, PE rounds
    # on ingest; the BIR verifier requires fp32r-typed producers) or fp16
    # (host-cast, gets fast-weight-load on the PE).
    # st layout [it, j%128, jt, i]: per-partition-contiguous for the slab DMA.
    st_dt = f16 if _OPTS["st16"] else f32
    st = nc.dram_tensor("st", [IT, P, JT * P], st_dt, kind="ExternalInput").ap()
    xg = nc.dram_tensor("xg", [CHUNK, HIDDEN], mm_dt, kind="ExternalInput").ap()
    out = nc.dram_tensor("out", [CHUNK, HIDDEN], f32, kind="ExternalOutput").ap()

    with tile.TileContext(nc) as tc:
        with (
            tc.tile_pool(name="xpool", bufs=1) as xpool,
            tc.tile_pool(name="spool", bufs=_OPTS["pool_bufs"]) as spool,
            tc.tile_pool(name="ppool", bufs=_OPTS["pool_bufs"]) as ppool,
            tc.tile_pool(name="opool", bufs=2) as opool,
            tc.tile_pool(name="cpool", bufs=1) as cpool,
            tc.tile_pool(name="psum_o", bufs=_OPTS["psum_o"], space="PSUM") as psum_o,
            tc.tile_pool(name="psum_s", bufs=_OPTS["psum_s"], space="PSUM") as psum_s,
        ):
            # fp32r matmuls need a moving free dim >= 2, so the rowsum "ones"
            # vector is two columns wide; for f32r it is built via exp(0*x)
            # since neither memset nor DMA can emit the f32r dtype the
            # verifier wants.
            if mm_dt == f32r:
                zeros_t = cpool.tile([P, 2], f32, name="zeros_t", tag="zeros")
                nc.vector.memset(zeros_t[:], 0.0)
                ones_t = cpool.tile([P, 2], f32r, name="ones_t", tag="ones")
                nc.scalar.activation(
                    ones_t[:], zeros_t[:], mybir.ActivationFunctionType.Exp,
                    scale=0.0,
                )
            else:
                ones_t = cpool.tile([P, 2], mm_dt, name="ones_t", tag="ones")
                nc.vector.memset(ones_t[:], 1.0)

            # xg stays resident in SBUF: 16 tiles of [128, 2048] (128 KiB/partition)
            xg_tiles = []

            def load_xg():
                xg_tiles.clear()
                for jt in range(JT):
                    xt = xpool.tile(
                        [P, HIDDEN], mm_dt, name=f"xg_{jt}", tag=f"xg_{jt}"
                    )
                    nc.sync.dma_start(xt[:], xg[jt * P : (jt + 1) * P, :])
                    xg_tiles.append(xt)

            load_xg()

            schedule = []
            for rep in range(reps):
                for it in range(IT):
                    schedule.append((rep, it))
            for rep, it in schedule:
                if reload_xg and it == 0 and rep > 0:
                    load_xg()
                # load scores^T slab for this row block: [j-part, (jt, i)].
                # ACT's HWDGE queue, so it isn't serialized behind the 16.8MB
                # xg load on SP's queue at kernel start.
                st_t = spool.tile([P, JT * P], st_dt, name="st_t", tag="st")
                dma_eng = nc.scalar if _OPTS["st_queue"] == "act" else nc.sync
                dma_eng.dma_start(st_t[:], st[it])
                # P^T = exp(scores^T / sqrt(head_dim)), rounded to mm_dt by ACT
                pt_t = ppool.tile([P, JT * P], mm_dt, name="pt_t", tag="pt")
                nc.scalar.activation(
                    pt_t[:], st_t[:], mybir.ActivationFunctionType.Exp, scale=SCALE
                )

                ps_s = psum_s.tile([P, 2], f32, name="ps_s", tag="ps_s")
                ps_tiles = [
                    psum_o.tile([P, NDW], f32, name=f"ps_o{db}", tag="ps_o")
                    for db in range(ND)
                ]
                for jt in range(JT):
                    lhsT = pt_t[:, jt * P : (jt + 1) * P]
                    start = jt == 0
                    stop = jt == JT - 1
                    for db in range(ND):
                        nc.tensor.matmul(
                            ps_tiles[db][:],
                            lhsT,
                            xg_tiles[jt][:, db * NDW : (db + 1) * NDW],
                            start=start,
                            stop=stop,
                        )
                    if not _OPTS["no_ones"]:
                        nc.tensor.matmul(
                            ps_s[:],
                            lhsT,
                            ones_t[:],
                            start=start,
                            stop=stop,
                        )

                rec = cpool.tile([P, 1], f32, name="rec", tag="rec", bufs=2)
                nc.vector.reciprocal(rec[:], ps_s[:, 0:1])
                ot = opool.tile([P, HIDDEN], f32, name="ot", tag="ot")
                for db in range(ND):
                    nc.vector.tensor_scalar_mul(
                        ot[:, db * NDW : (db + 1) * NDW], ps_tiles[db][:], rec[:]
                    )
                nc.sync.dma_start(out[it * P : (it + 1) * P, :], ot[:])

    nc.compile()
    return nc


def _build_nc_merged(reps: int = 1):
    """Merged-rings algorithm: sum_r softmax_r @ X == (sum_r softmax_r) @ X.

    Cores = 2 batches x 4 j-quarters. Each core exps all 4 rings in natural
    layout (accum_out gives the softmax row-sums for free), builds the merged
    normalized W for its j-quarter (columns 0-511 after a host-side column
    rotation), PE-transposes W, and matmuls against its 512-row slice of xg.
    Host sums the 4 quarter-partials per batch. 4x fewer matmul FLOPs than
    the per-ring scheme.
    """
    import concourse.mybir as mybir
    import concourse.tile as tile
    from concourse import bacc
    from concourse.masks import make_identity

    f32 = mybir.dt.float32
    f16 = mybir.dt.float16
    QW = 512  # j-quarter width
    QT = QW // P  # 4 contraction tiles

    nc = bacc.Bacc(
        "TRN2", target_bir_lowering=False, debug=False, num_devices=NCORES
    )
    stn = nc.dram_tensor(
        "stn", [RING, IT, P, CHUNK], f16, kind="ExternalInput"
    ).ap()
    xgq = nc.dram_tensor("xgq", [QW, HIDDEN], f16, kind="ExternalInput").ap()
    out_dt = f16 if _OPTS["out16"] else f32
    out = nc.dram_tensor("out", [CHUNK, HIDDEN], out_dt, kind="ExternalOutput").ap()

    with tile.TileContext(nc) as tc:
        with (
            tc.tile_pool(name="xpool", bufs=1) as xpool,
            tc.tile_pool(name="spool", bufs=3) as spool,
            tc.tile_pool(name="epool", bufs=2) as epool,
            tc.tile_pool(name="wpool", bufs=2) as wpool,
            tc.tile_pool(name="wtpool", bufs=2) as wtpool,
            tc.tile_pool(name="opool", bufs=2) as opool,
            tc.tile_pool(name="cpool", bufs=1) as cpool,
            tc.tile_pool(name="sapool", bufs=8) as sapool,
            tc.tile_pool(name="psum_o", bufs=6, space="PSUM") as psum_o,
            tc.tile_pool(name="psum_t", bufs=2, space="PSUM") as psum_t,
        ):
            ident = cpool.tile([P, P], f16, name="ident", tag="ident")
            make_identity(nc, ident[:])

            xg_tiles = []
            for jt in range(QT):
                xt = xpool.tile([P, HIDDEN], f16, name=f"xgq_{jt}", tag=f"xgq_{jt}")
                nc.sync.dma_start(xt[:], xgq[jt * P : (jt + 1) * P, :])
                xg_tiles.append(xt)

            for it in [i for _ in range(reps) for i in range(IT)]:
                # all 4 ring slabs DMA'd from the SP queue up front so the ACT
                # stream is pure back-to-back exps (ACT is the bottleneck)
                st_ts = []
                for r in range(RING):
                    st_t = spool.tile(
                        [P, CHUNK], f16, name=f"st_t{r}", tag=f"st{r}"
                    )
                    nc.sync.dma_start(st_t[:], stn[r, it])
                    st_ts.append(st_t)
                sa4 = sapool.tile([P, RING], f32, name="sa4", tag="sa4")
                e_ts = []
                for r in range(RING):
                    e_t = epool.tile([P, CHUNK], f16, name=f"e_t{r}", tag=f"e{r}")
                    nc.scalar.activation(
                        e_t[:], st_ts[r][:], mybir.ActivationFunctionType.Exp,
                        scale=SCALE, accum_out=sa4[:, r : r + 1],
                    )
                    e_ts.append(e_t)
                rec4 = sapool.tile([P, RING], f32, name="rec4", tag="rec4")
                nc.vector.reciprocal(rec4[:], sa4[:])
                w_t = wpool.tile([P, QW], f16, name="w_t", tag="w")
                for r in range(RING):
                    rec_r = rec4[:, r : r + 1]
                    if r == 0:
                        nc.vector.tensor_scalar_mul(
                            w_t[:], e_ts[r][:, 0:QW], rec_r
                        )
                    else:
                        tmp = wpool.tile([P, QW], f16, name="tmp", tag="tmp")
                        nc.vector.tensor_scalar_mul(
                            tmp[:], e_ts[r][:, 0:QW], rec_r
                        )
                        nc.vector.tensor_tensor(
                            w_t[:], w_t[:], tmp[:], op=mybir.AluOpType.add
                        )

                # W^T tiles for the matmul stationary side
                wt_tiles = []
                for jt in range(QT):
                    pst = psum_t.tile([P, P], f16, name="pst", tag="pst")
                    nc.tensor.transpose(
                        pst[:], w_t[:, jt * P : (jt + 1) * P], ident[:]
                    )
                    wt = wtpool.tile([P, P], f16, name=f"wt{jt}", tag=f"wt{jt}")
                    nc.vector.tensor_copy(wt[:], pst[:])
                    wt_tiles.append(wt)

                ps_tiles = [
                    psum_o.tile([P, NDW], f32, name=f"ps_o{db}", tag="ps_o")
                    for db in range(ND)
                ]
                for jt in range(QT):
                    for db in range(ND):
                        nc.tensor.matmul(
                            ps_tiles[db][:],
                            wt_tiles[jt][:],
                            xg_tiles[jt][:, db * NDW : (db + 1) * NDW],
                            start=(jt == 0),
                            stop=(jt == QT - 1),
                        )
                ot = opool.tile([P, HIDDEN], out_dt, name="ot", tag="ot")
                for db in range(ND):
                    # DVE, not ACT: the four exp passes saturate ACT
                    nc.vector.tensor_copy(
                        ot[:, db * NDW : (db + 1) * NDW], ps_tiles[db][:]
                    )
                nc.sync.dma_start(out[it * P : (it + 1) * P, :], ot[:])

    nc.compile()
    return nc


def _prep_inputs_ring(x, scores, idx):
    xg = x[:, idx, :]
    if _OPTS["dtype"] == "fp16":
        xg = xg.astype(np.float16)
    else:
        xg = np.ascontiguousarray(xg)
    # blocked transpose of scores for j-major device layout:
    # stb[r, b, it, p, jt*128+i] = scores[r, b, it*128+i, jt*128+p]
    stb_t = scores.reshape(RING, BATCH, IT, P, JT, P).transpose(0, 1, 2, 5, 4, 3)
    stb = stb_t.astype(np.float16 if _OPTS["st16"] else np.float32).reshape(
        RING, BATCH, IT, P, JT * P
    )
    return [{"st": stb[c // 2, c % 2], "xg": xg[c % 2]} for c in range(NCORES)]


def _prep_inputs_merged(x, scores, idx):
    QW = 512
    xg = x[:, idx, :]
    in_maps = []
    for c in range(NCORES):
        b, q = c // 4, c % 4
        s_b = scores[:, b]  # [4, 2048, 2048]
        # rotate j so this core's quarter occupies columns 0..511 (softmax is
        # column-order invariant; the matmul only touches the first quarter)
        stn = np.empty((RING, CHUNK, CHUNK), np.float16)
        k = q * QW
        stn[..., : CHUNK - k] = s_b[..., k:]
        if k:
            stn[..., CHUNK - k :] = s_b[..., :k]
        in_maps.append({
            "stn": stn.reshape(RING, IT, P, CHUNK),
            "xgq": xg[b][k : k + QW].astype(np.float16),
        })
    return in_maps


def kernel(**inputs) -> np.ndarray:
    from concourse import bass_utils

    x = np.asarray(inputs["x"], dtype=np.float32)
    scores = np.asarray(inputs["scores"], dtype=np.float32)

    idx = _hilbert_idx()
    merged = _OPTS["algo"] == "merged"
    if "nc" not in _CACHE:
        _CACHE["nc"] = _build_nc_merged() if merged else _build_nc()
    nc = _CACHE["nc"]

    in_maps = (
        _prep_inputs_merged(x, scores, idx)
        if merged
        else _prep_inputs_ring(x, scores, idx)
    )
    res = bass_utils.run_bass_kernel_spmd(
        nc, in_maps, core_ids=list(range(NCORES)), trace=_TRACE
    )
    if _TRACE:
        global _LAST_EXEC_NS
        _LAST_EXEC_NS = res.exec_time_ns

    acc = np.zeros((BATCH, CHUNK, HIDDEN), dtype=np.float32)
    for c in range(NCORES):
        acc[c // 4 if merged else c % 2] += res.results[c]["out"]

    out_full = np.zeros((BATCH, SEQ, HIDDEN), dtype=np.float32)
    out_full[:, idx, :] = acc
    return out_full


# revision 50
# speedup vs baseline: 1.7000x; 1.0769x over previous
"""Trainium2 Bass kernel for MockHilbertRingAttention.

Math (from the reference):
  mapping = snake-scan permutation of [0, 8192)
  idx     = mapping[0:2048]                (RANK=0, chunk=2048)
  xg[b]   = x[b, idx, :]                      [2, 2048, 2048]
  P[r,b]  = softmax(scores[r,b] / sqrt(128), axis=-1)
  acc[b]  = sum_r P[r,b] @ xg[b]              [2, 2048, 2048]
  out     = zeros(2, 8192, 2048); out[:, idx, :] = acc

Distribution: 8 cores = 4 rings x 2 batches; core c handles (r=c//2, b=c%2).
Each core computes  out_c = softmax(scores[r,b]/T) @ xg[b]  (normalized), the
host sums the 4 ring partials per batch and scatters rows back.

Device kernel per core (2048x2048x2048 fp32 matmul + softmax):
  - scores are fed pre-transposed (j-major) so the exp'd tile can be used
    directly as the matmul stationary operand [K=j, M=i]
  - row sums of exp come from an extra N=1 matmul against a ones vector,
    accumulated in PSUM alongside the main output
  - normalization (1/rowsum) is applied per-partition while draining PSUM
  - matmuls run in float32r mode (full fp32 data, 1 cycle/row at N=512)
"""

import numpy as np

P = 128
CHUNK = 2048
SEQ = 8192
HIDDEN = 2048
RING = 4
BATCH = 2
NCORES = 8
IT = CHUNK // P  # 16 output row blocks
JT = CHUNK // P  # 16 contraction blocks
ND = 4  # 4 moving blocks of 512 over hidden dim
NDW = HIDDEN // ND  # 512
SCALE = float(1.0 / np.sqrt(128.0))

_CACHE = {}
_TRACE = False  # set by test.py to capture HW exec time via NTFF profiling
_LAST_EXEC_NS = None
# build-time knobs for A/B benchmarking (bench.py overrides).
# fp16 matmul operands: ~10% faster than f32r (fast-weight-load on the PE,
# half the DMA bytes) at rel err 2.4e-4 vs 1.2e-4.
_OPTS = {
    "st_queue": "act", "psum_o": 7, "psum_s": 1, "dtype": "fp16",
    "st16": True, "pool_bufs": 2, "no_ones": False, "algo": "merged",
    "out16": True,
}


def _hilbert_idx() -> np.ndarray:
    """First CHUNK entries of the snake-scan mapping (mapping[pos] = scan idx)."""
    size = SEQ
    grid = int(np.ceil(np.sqrt(size)))
    order = []
    for row in range(grid):
        cols = range(grid) if row % 2 == 0 else range(grid - 1, -1, -1)
        for col in cols:
            pos = row * grid + col
            if pos < size:
                order.append(pos)
    mapping = np.zeros(size, dtype=np.int64)
    mapping[np.asarray(order)] = np.arange(size, dtype=np.int64)
    return mapping[:CHUNK]


def _build_nc(reps: int = 1, reload_xg: bool = False):
    # reps>1 repeats the whole compute body (bench-only: isolates steady-state
    # device throughput from per-dispatch overhead). reload_xg puts the xg
    # resident-load inside the rep loop so each rep pays the pipe-fill the
    # real single-shot invocation pays.
    import concourse.mybir as mybir
    import concourse.tile as tile
    from concourse import bacc

    f32 = mybir.dt.float32
    f32r = mybir.dt.float32r
    f16 = mybir.dt.float16
    mm_dt = f16 if _OPTS["dtype"] == "fp16" else f32r

    nc = bacc.Bacc(
        "TRN2", target_bir_lowering=False, debug=False, num_devices=NCORES
    )
    # xg is declared in the matmul dtype: f32r (same bits as fp32, PE rounds
    # on ingest; the BIR verifier requires fp32r-typed producers) or fp16
    # (host-cast, gets fast-weight-load on the PE).
    # st layout [it, j%128, jt, i]: per-partition-contiguous for the slab DMA.
    st_dt = f16 if _OPTS["st16"] else f32
    st = nc.dram_tensor("st", [IT, P, JT * P], st_dt, kind="ExternalInput").ap()
    xg = nc.dram_tensor("xg", [CHUNK, HIDDEN], mm_dt, kind="ExternalInput").ap()
    out = nc.dram_tensor("out", [CHUNK, HIDDEN], f32, kind="ExternalOutput").ap()

    with tile.TileContext(nc) as tc:
        with (
            tc.tile_pool(name="xpool", bufs=1) as xpool,
            tc.tile_pool(name="spool", bufs=_OPTS["pool_bufs"]) as spool,
            tc.tile_pool(name="ppool", bufs=_OPTS["pool_bufs"]) as ppool,
            tc.tile_pool(name="opool", bufs=2) as opool,
            tc.tile_pool(name="cpool", bufs=1) as cpool,
            tc.tile_pool(name="psum_o", bufs=_OPTS["psum_o"], space="PSUM") as psum_o,
            tc.tile_pool(name="psum_s", bufs=_OPTS["psum_s"], space="PSUM") as psum_s,
        ):
            # fp32r matmuls need a moving free dim >= 2, so the rowsum "ones"
            # vector is two columns wide; for f32r it is built via exp(0*x)
            # since neither memset nor DMA can emit the f32r dtype the
            # verifier wants.
            if mm_dt == f32r:
                zeros_t = cpool.tile([P, 2], f32, name="zeros_t", tag="zeros")
                nc.vector.memset(zeros_t[:], 0.0)
                ones_t = cpool.tile([P, 2], f32r, name="ones_t", tag="ones")
                nc.scalar.activation(
                    ones_t[:], zeros_t[:], mybir.ActivationFunctionType.Exp,
                    scale=0.0,
                )
            else:
                ones_t = cpool.tile([P, 2], mm_dt, name="ones_t", tag="ones")
                nc.vector.memset(ones_t[:], 1.0)

            # xg stays resident in SBUF: 16 tiles of [128, 2048] (128 KiB/partition)
            xg_tiles = []

            def load_xg():
                xg_tiles.clear()
                for jt in range(JT):
                    xt = xpool.tile(
                        [P, HIDDEN], mm_dt, name=f"xg_{jt}", tag=f"xg_{jt}"
                    )
                    nc.sync.dma_start(xt[:], xg[jt * P : (jt + 1) * P, :])
                    xg_tiles.append(xt)

            load_xg()

            schedule = []
            for rep in range(reps):
                for it in range(IT):
                    schedule.append((rep, it))
            for rep, it in schedule:
                if reload_xg and it == 0 and rep > 0:
                    load_xg()
                # load scores^T slab for this row block: [j-part, (jt, i)].
                # ACT's HWDGE queue, so it isn't serialized behind the 16.8MB
                # xg load on SP's queue at kernel start.
                st_t = spool.tile([P, JT * P], st_dt, name="st_t", tag="st")
                dma_eng = nc.scalar if _OPTS["st_queue"] == "act" else nc.sync
                dma_eng.dma_start(st_t[:], st[it])
                # P^T = exp(scores^T / sqrt(head_dim)), rounded to mm_dt by ACT
                pt_t = ppool.tile([P, JT * P], mm_dt, name="pt_t", tag="pt")
                nc.scalar.activation(
                    pt_t[:], st_t[:], mybir.ActivationFunctionType.Exp, scale=SCALE
                )

                ps_s = psum_s.tile([P, 2], f32, name="ps_s", tag="ps_s")
                ps_tiles = [
                    psum_o.tile([P, NDW], f32, name=f"ps_o{db}", tag="ps_o")
                    for db in range(ND)
                ]
                for jt in range(JT):
                    lhsT = pt_t[:, jt * P : (jt + 1) * P]
                    start = jt == 0
                    stop = jt == JT - 1
                    for db in range(ND):
                        nc.tensor.matmul(
                            ps_tiles[db][:],
                            lhsT,
                            xg_tiles[jt][:, db * NDW : (db + 1) * NDW],
                            start=start,
                            stop=stop,
                        )
                    if not _OPTS["no_ones"]:
                        nc.tensor.matmul(
                            ps_s[:],
                            lhsT,
                            ones_t[:],
                            start=start,
                            stop=stop,
                        )

                rec = cpool.tile([P, 1], f32, name="rec", tag="rec", bufs=2)
                nc.vector.reciprocal(rec[:], ps_s[:, 0:1])
                ot = opool.tile([P, HIDDEN], f32, name="ot", tag="ot")
                for db in range(ND):
                    nc.vector.tensor_scalar_mul(
                        ot[:, db * NDW : (db + 1) * NDW], ps_tiles[db][:], rec[:]
                    )
                nc.sync.dma_start(out[it * P : (it + 1) * P, :], ot[:])

    nc.compile()
    return nc


def _build_nc_merged(reps: int = 1):
    """Merged-rings algorithm: sum_r softmax_r @ X == (sum_r softmax_r) @ X.

    Cores = 2 batches x 4 i-row-quarters. Each core exps all 4 rings for its
    own 512 output rows only (natural layout; accum_out gives the softmax
    row-sums locally — full j per row, so no cross-core dependency), builds
    the merged normalized W [512, 2048], PE-transposes it, and matmuls
    against the full xg. Host CONCATENATES the row-quarters per batch.
    4x fewer matmul FLOPs and 4x less exp work than the per-ring scheme.
    """
    import concourse.mybir as mybir
    import concourse.tile as tile
    from concourse import bacc
    from concourse.masks import make_identity

    f32 = mybir.dt.float32
    f16 = mybir.dt.float16
    ITQ = 4  # 4 row-blocks of 128 = this core's 512-row quarter
    QT = JT  # contract over all 16 j-tiles

    nc = bacc.Bacc(
        "TRN2", target_bir_lowering=False, debug=False, num_devices=NCORES
    )
    stn = nc.dram_tensor(
        "stn", [RING, ITQ, P, CHUNK], f16, kind="ExternalInput"
    ).ap()
    xgq = nc.dram_tensor("xgq", [CHUNK, HIDDEN], f16, kind="ExternalInput").ap()
    out_dt = f16 if _OPTS["out16"] else f32
    out = nc.dram_tensor(
        "out", [ITQ * P, HIDDEN], out_dt, kind="ExternalOutput"
    ).ap()

    with tile.TileContext(nc) as tc:
        with (
            tc.tile_pool(name="xpool", bufs=1) as xpool,
            tc.tile_pool(name="spool", bufs=3) as spool,
            tc.tile_pool(name="epool", bufs=2) as epool,
            tc.tile_pool(name="wpool", bufs=2) as wpool,
            tc.tile_pool(name="wtpool", bufs=2) as wtpool,
            tc.tile_pool(name="opool", bufs=2) as opool,
            tc.tile_pool(name="cpool", bufs=1) as cpool,
            tc.tile_pool(name="sapool", bufs=8) as sapool,
            tc.tile_pool(name="psum_o", bufs=6, space="PSUM") as psum_o,
            tc.tile_pool(name="psum_t", bufs=2, space="PSUM") as psum_t,
        ):
            ident = cpool.tile([P, P], f16, name="ident", tag="ident")
            make_identity(nc, ident[:])

            xg_tiles = []
            for jt in range(QT):
                xt = xpool.tile([P, HIDDEN], f16, name=f"xgq_{jt}", tag=f"xgq_{jt}")
                nc.sync.dma_start(xt[:], xgq[jt * P : (jt + 1) * P, :])
                xg_tiles.append(xt)

            for it in [i for _ in range(reps) for i in range(ITQ)]:
                # all 4 ring slabs DMA'd from the SP queue up front so the ACT
                # stream is pure back-to-back exps
                st_ts = []
                for r in range(RING):
                    st_t = spool.tile(
                        [P, CHUNK], f16, name=f"st_t{r}", tag=f"st{r}"
                    )
                    nc.sync.dma_start(st_t[:], stn[r, it])
                    st_ts.append(st_t)
                sa4 = sapool.tile([P, RING], f32, name="sa4", tag="sa4")
                e_ts = []
                for r in range(RING):
                    e_t = epool.tile([P, CHUNK], f16, name=f"e_t{r}", tag=f"e{r}")
                    nc.scalar.activation(
                        e_t[:], st_ts[r][:], mybir.ActivationFunctionType.Exp,
                        scale=SCALE, accum_out=sa4[:, r : r + 1],
                    )
                    e_ts.append(e_t)
                rec4 = sapool.tile([P, RING], f32, name="rec4", tag="rec4")
                nc.vector.reciprocal(rec4[:], sa4[:])
                w_t = wpool.tile([P, CHUNK], f16, name="w_t", tag="w")
                for r in range(RING):
                    rec_r = rec4[:, r : r + 1]
                    if r == 0:
                        nc.vector.tensor_scalar_mul(w_t[:], e_ts[r][:], rec_r)
                    else:
                        tmp = wpool.tile([P, CHUNK], f16, name="tmp", tag="tmp")
                        nc.vector.tensor_scalar_mul(tmp[:], e_ts[r][:], rec_r)
                        nc.vector.tensor_tensor(
                            w_t[:], w_t[:], tmp[:], op=mybir.AluOpType.add
                        )

                # W^T tiles for the matmul stationary side
                wt_tiles = []
                for jt in range(QT):
                    pst = psum_t.tile([P, P], f16, name="pst", tag="pst")
                    nc.tensor.transpose(
                        pst[:], w_t[:, jt * P : (jt + 1) * P], ident[:]
                    )
                    wt = wtpool.tile([P, P], f16, name=f"wt{jt}", tag=f"wt{jt}")
                    nc.vector.tensor_copy(wt[:], pst[:])
                    wt_tiles.append(wt)

                ps_tiles = [
                    psum_o.tile([P, NDW], f32, name=f"ps_o{db}", tag="ps_o")
                    for db in range(ND)
                ]
                for jt in range(QT):
                    for db in range(ND):
                        nc.tensor.matmul(
                            ps_tiles[db][:],
                            wt_tiles[jt][:],
                            xg_tiles[jt][:, db * NDW : (db + 1) * NDW],
                            start=(jt == 0),
                            stop=(jt == QT - 1),
                        )
                ot = opool.tile([P, HIDDEN], out_dt, name="ot", tag="ot")
                for db in range(ND):
                    # DVE, not ACT: the four exp passes saturate ACT
                    nc.vector.tensor_copy(
                        ot[:, db * NDW : (db + 1) * NDW], ps_tiles[db][:]
                    )
                nc.sync.dma_start(out[it * P : (it + 1) * P, :], ot[:])

    nc.compile()
    return nc


def _prep_inputs_ring(x, scores, idx):
    xg = x[:, idx, :]
    if _OPTS["dtype"] == "fp16":
        xg = xg.astype(np.float16)
    else:
        xg = np.ascontiguousarray(xg)
    # blocked transpose of scores for j-major device layout:
    # stb[r, b, it, p, jt*128+i] = scores[r, b, it*128+i, jt*128+p]
    stb_t = scores.reshape(RING, BATCH, IT, P, JT, P).transpose(0, 1, 2, 5, 4, 3)
    stb = stb_t.astype(np.float16 if _OPTS["st16"] else np.float32).reshape(
        RING, BATCH, IT, P, JT * P
    )
    return [{"st": stb[c // 2, c % 2], "xg": xg[c % 2]} for c in range(NCORES)]


def _prep_inputs_merged(x, scores, idx):
    QH = 512  # i-row quarter height
    xg = x[:, idx, :]
    xg16 = [xg[b].astype(np.float16) for b in range(BATCH)]
    in_maps = []
    for c in range(NCORES):
        b, m = c // 4, c % 4
        stn = scores[:, b, m * QH : (m + 1) * QH, :].astype(np.float16)
        in_maps.append({
            "stn": stn.reshape(RING, 4, P, CHUNK),
            "xgq": xg16[b],
        })
    return in_maps


def kernel(**inputs) -> np.ndarray:
    from concourse import bass_utils

    x = np.asarray(inputs["x"], dtype=np.float32)
    scores = np.asarray(inputs["scores"], dtype=np.float32)

    idx = _hilbert_idx()
    merged = _OPTS["algo"] == "merged"
    if "nc" not in _CACHE:
        _CACHE["nc"] = _build_nc_merged() if merged else _build_nc()
    nc = _CACHE["nc"]

    in_maps = (
        _prep_inputs_merged(x, scores, idx)
        if merged
        else _prep_inputs_ring(x, scores, idx)
    )
    res = bass_utils.run_bass_kernel_spmd(
        nc, in_maps, core_ids=list(range(NCORES)), trace=_TRACE
    )
    if _TRACE:
        global _LAST_EXEC_NS
        _LAST_EXEC_NS = res.exec_time_ns

    acc = np.zeros((BATCH, CHUNK, HIDDEN), dtype=np.float32)
    for c in range(NCORES):
        if merged:
            b, m = c // 4, c % 4
            acc[b, m * 512 : (m + 1) * 512] = res.results[c]["out"]
        else:
            acc[c % 2] += res.results[c]["out"]

    out_full = np.zeros((BATCH, SEQ, HIDDEN), dtype=np.float32)
    out_full[:, idx, :] = acc
    return out_full
